# revision 16
# baseline (speedup 1.0000x reference)
"""Trainium2 Bass kernel for nn_AudioDeviceModel (dilated causal conv stack).

Strategy:
  - Data parallel: batch 64 sharded as 8 rows per core across 8 cores.
  - Only the last FRAME=128 timesteps are output; the receptive field of the
    10-layer dilated conv stack (dilations 1..512, K=3) is 2047, so only the
    last 2174 input samples matter.  Per-layer output windows shrink
    accordingly (W_Y below).
  - On-chip layout: partitions = (batch b in 0..8) x (channel c in 0..8) = 64
    partitions; time on the free axis.  Convs become PSUM-accumulated matmuls
    with host-built block-diagonal weights kron(eye(8), W).
  - All matmul operands use float32r (1 cycle/col on the PE at N>=256 vs 4+
    cycles for fp32; ~11-bit mantissa).  To stop rounding error compounding
    down the residual chain, h_i is carried in exact fp32 (H tensors, VectorE
    adds) and only rounded f32r copies feed the matmuls.  io biases are
    folded into later conv biases on the host so the residual add is a
    single 2-input DVE op.
  - Tap stacking: A_i [128, W_H[i]] f32r holds round(h_i) in rows 0:64 and
    the same values shifted right by dil in rows 64:128, so conv taps 0+1
    run as ONE K=128 matmul and tap 2 as a K=64 matmul (2N instead of 3N
    PE cycles per tile).  Layer 0 stacks all 3 taps of the 1-channel input
    into one K=24 matmul (x stored 3x with shifts in rows 0:24).
  - Residual: 1 matmul (U_i blockdiag, y -> psum) + DVE add with H_i ->
    H_{i+1}, then two DVE round-casts produce A_{i+1} rows 0:64 and the
    shifted dup rows 64:128.
  - Mixer: 10 accumulated [64,8]x[64,128] matmuls interleaved at each
    layer's end (skip_group_check) + bias.
"""

import sys

import numpy as np

try:
    import concourse.bass as bass
except ImportError:  # fresh environment without the site path
    sys.path.insert(0, "/opt/trn_rl_repo")
    import concourse.bass as bass

import concourse.tile as tile
from concourse import bacc, mybir
from concourse.bass_utils import run_bass_kernel_spmd

N_LAYERS = 10
FRAME = 128
B, T = 64, 4096
N_CORES = 8
B_LOC = B // N_CORES  # 8 batch rows per core
NT = 512  # time-tile (one PSUM bank of f32)

# per-layer dilations and windows
DIL = [2**i for i in range(N_LAYERS)]
W_Y = [0] * N_LAYERS  # output window of layer i (cols of y_i computed)
W_H = [0] * N_LAYERS  # input window of layer i (cols of h_i needed)
W_Y[N_LAYERS - 1] = FRAME
for _i in range(N_LAYERS - 1, -1, -1):
    W_H[_i] = W_Y[_i] + 2 * DIL[_i]
    if _i > 0:
        W_Y[_i - 1] = W_H[_i]
W_X = W_H[0]  # 2174

_F32 = mybir.dt.float32
_F32R = mybir.dt.float32r


def _tiles(wy):
    """End-aligned tiling: (start, size) pairs — ragged first tile, then
    512-wide tiles."""
    r = wy % NT
    starts = ([0] if r else []) + list(range(r, wy, NT))
    return [(s, (starts[k + 1] if k + 1 < len(starts) else wy) - s)
            for k, s in enumerate(starts)]


def _build_program():
    """Build the Bass program once; weights/x arrive as DRAM inputs."""
    nc = bacc.Bacc(
        "TRN2",
        target_bir_lowering=False,
        debug=False,
        enable_asserts=True,
        num_devices=N_CORES,
    )

    d_x = nc.dram_tensor("xw", [B_LOC, W_X], _F32, kind="ExternalInput").ap()
    d_w0 = nc.dram_tensor("w0", [24, 64], _F32, kind="ExternalInput").ap()
    d_wc = nc.dram_tensor("wc", [128, 9 * 128], _F32, kind="ExternalInput").ap()
    d_wr = nc.dram_tensor("wr", [64, 640], _F32, kind="ExternalInput").ap()
    d_wm = nc.dram_tensor("wm", [64, 80], _F32, kind="ExternalInput").ap()
    d_cb = nc.dram_tensor("cb", [64, N_LAYERS], _F32, kind="ExternalInput").ap()
    d_mb = nc.dram_tensor("mb", [8, 1], _F32, kind="ExternalInput").ap()
    d_out = nc.dram_tensor("out", [B_LOC, FRAME], _F32, kind="ExternalOutput").ap()

    with tile.TileContext(nc) as tc:
        with (
            tc.tile_pool(name="wpool", bufs=1) as wpool,
            tc.tile_pool(name="apool", bufs=2) as apool,
            tc.tile_pool(name="hpool", bufs=2) as hpool,
            tc.tile_pool(name="ypool", bufs=4) as ypool,
            tc.tile_pool(name="opool", bufs=1) as opool,
            tc.tile_pool(name="py", bufs=1, space="PSUM") as pyp,
            tc.tile_pool(name="ph", bufs=2, space="PSUM") as php,
            tc.tile_pool(name="pm", bufs=1, space="PSUM") as pmp,
        ):
            # --- inputs/weights: DMA to fp32 staging, round-copy to f32r.
            # Emission order matters: DVE runs casts in order, so cast the
            # things layer 0 needs FIRST (x, layer-0 taps, residual weights)
            # and the big conv-weight block last, overlapping compute.
            # x triplicated with shifts into rows 0:24 via 3 DMAs so conv0
            # is ONE K=24 matmul per tile (col j0+2+j: rows 0:8 tap2,
            # 8:16 tap1, 16:24 tap0)
            XS3 = opool.tile([24, W_X], _F32, tag="XS3", name="XS3")
            nc.sync.dma_start(XS3[0:8, :], d_x[:, :])
            nc.sync.dma_start(XS3[8:16, 1:W_X], d_x[:, 0 : W_X - 1])
            nc.sync.dma_start(XS3[16:24, 2:W_X], d_x[:, 0 : W_X - 2])
            WHs = wpool.tile([128, 192 + 9 * 128], _F32, tag="WHs", name="WHs")
            WH = wpool.tile([128, 192 + 9 * 128], _F32R, tag="WH", name="WH")
            nc.sync.dma_start(WHs[0:24, 0:64], d_w0[:, :])
            nc.vector.tensor_copy(WH[0:24, 0:64], WHs[0:24, 0:64])
            CB = wpool.tile([64, N_LAYERS], _F32, tag="CB", name="CB")
            nc.sync.dma_start(CB[:, :], d_cb[:, :])
            MB = wpool.tile([8, 1], _F32, tag="MB", name="MB")
            nc.sync.dma_start(MB[:, :], d_mb[:, :])
            # residual weights: cols 0:576 U_i blockdiag (rows 0:64);
            # cols 576:640 x-broadcast (rows 0:8)
            WRs = wpool.tile([64, 640], _F32, tag="WRs", name="WRs")
            WR = wpool.tile([64, 640], _F32R, tag="WR", name="WR")
            nc.sync.dma_start(WRs[:, :], d_wr[:, :])
            nc.vector.tensor_copy(WR[:, 0:576], WRs[:, 0:576])
            nc.vector.tensor_copy(WR[0:8, 576:640], WRs[0:8, 576:640])
            # big conv-weight block: split DMA across two queues
            half = 192 + 4 * 128
            nc.sync.dma_start(WHs[:, 192:half], d_wc[:, 0 : half - 192])
            nc.sync.dma_start(WHs[:, half:], d_wc[:, half - 192 :])
            nc.vector.tensor_copy(WH[:, 192:half], WHs[:, 192:half])
            nc.vector.tensor_copy(WH[:, half:], WHs[:, half:])
            # mixer weights rows 0:64
            WMs = wpool.tile([64, 80], _F32, tag="WMs", name="WMs")
            nc.sync.dma_start(WMs[:, :], d_wm[:, :])
            WM = wpool.tile([64, 80], _F32R, tag="WM", name="WM")
            nc.vector.tensor_copy(WM[:, :], WMs[:, :])

            # --- activation tensors ---
            # A_i [128, W_H[i]] f32r: rows 0:64 round(h_i); rows 64:128 the
            # same shifted right by DIL[i] (for the stacked taps-(1,0)
            # matmul).  A_0: x in rows 0:8, x>>1 rows 8:16, x>>2 rows 16:24.
            # H_i [64, W_H[i]] fp32 exact h chain.  A/H rotate (bufs=2).
            A = [None] * N_LAYERS
            H = [None] * N_LAYERS
            ylast = [None] * N_LAYERS
            A[0] = apool.tile([128, W_H[0]], _F32R, tag="A", name="A0")
            nc.vector.tensor_copy(A[0][0:24, 2:W_X], XS3[:, 2:W_X])

            pm = pmp.tile([8, FRAME], _F32, tag="pm", name="pm")

            for i in range(N_LAYERS):
                d = DIL[i]
                wy = W_Y[i]
                off = 2 * d  # h_i col offset for time-aligned reads
                if i < N_LAYERS - 1:
                    A[i + 1] = apool.tile(
                        [128, W_H[i + 1]], _F32R, tag="A", name=f"A{i+1}"
                    )
                    H[i + 1] = hpool.tile(
                        [64, W_H[i + 1]], _F32, tag="H", name=f"H{i+1}"
                    )
                tl = _tiles(wy)
                # k-major matmul order: consecutive MMs share lhsT (the PE
                # pays ~150ns per weight swap), using len(tl)<=5 psum banks
                pys = [
                    pyp.tile([64, n], _F32, tag=f"py{ti}", name=f"py_{i}_{j0}")
                    for ti, (j0, n) in enumerate(tl)
                ]
                if i == 0:
                    for (j0, n), py in zip(tl, pys):
                        nc.tensor.matmul(
                            py[:, :],
                            WH[0:24, 0:64],
                            A[0][0:24, j0 + 2 : j0 + 2 + n],
                            start=True,
                            stop=True,
                        )
                else:
                    c0 = 192 + (i - 1) * 128
                    for (j0, n), py in zip(tl, pys):
                        # taps 1+0 stacked: rows 0:64 h at col+d (tap1),
                        # rows 64:128 dup(h>>d) at col+d = h at col (tap0)
                        nc.tensor.matmul(
                            py[:, :],
                            WH[0:128, c0 : c0 + 64],
                            A[i][0:128, j0 + d : j0 + d + n],
                            start=True,
                            stop=False,
                        )
                    for (j0, n), py in zip(tl, pys):
                        nc.tensor.matmul(
                            py[:, :],
                            WH[0:64, c0 + 64 : c0 + 128],
                            A[i][0:64, j0 + 2 * d : j0 + 2 * d + n],
                            start=False,
                            stop=True,
                        )
                # relu pass (ACT)
                yts = []
                for ti, ((j0, n), py) in enumerate(zip(tl, pys)):
                    ytag = f"YL{i}" if ti == len(tl) - 1 else "Y"
                    yt = ypool.tile([64, n], _F32R, tag=ytag, name=f"Y_{i}_{j0}")
                    nc.scalar.activation(
                        yt[:, :],
                        py[:, :],
                        mybir.ActivationFunctionType.Relu,
                        bias=CB[:, i : i + 1],
                    )
                    yts.append(yt)
                    ylast[i] = (yt, n)
                if i < N_LAYERS - 1:
                    # residual matmuls (same U_i weights back-to-back)
                    phs = []
                    for (j0, n), yt in zip(tl, yts):
                        ph = php.tile([64, n], _F32, tag="ph", name=f"ph_{i}_{j0}")
                        nc.tensor.matmul(
                            ph[:, :],
                            WR[0:64, i * 64 : i * 64 + 64],
                            yt[:, :],
                            start=True,
                            stop=(i != 0),
                            skip_group_check=True,
                        )
                        phs.append(ph)
                    if i == 0:
                        for (j0, n), ph in zip(tl, phs):
                            nc.tensor.matmul(
                                ph[:, :],
                                WR[0:8, 576:640],
                                A[0][0:8, off + j0 : off + j0 + n],
                                start=False,
                                stop=True,
                                skip_group_check=True,
                            )
                    # epilogue per tile: exact-fp32 h chain + f32r casts
                    d1 = DIL[i + 1]
                    for (j0, n), ph in zip(tl, phs):
                        if i == 0:
                            nc.vector.tensor_copy(H[1][:, j0 : j0 + n], ph[:, :])
                        else:
                            nc.vector.tensor_add(
                                H[i + 1][:, j0 : j0 + n],
                                ph[:, :],
                                H[i][:, off + j0 : off + j0 + n],
                            )
                        nc.vector.tensor_copy(
                            A[i + 1][0:64, j0 : j0 + n], H[i + 1][:, j0 : j0 + n]
                        )
                        je = min(j0 + n, W_Y[i + 1])
                        if j0 < je:
                            nc.gpsimd.tensor_copy(
                                A[i + 1][64:128, j0 + d1 : je + d1],
                                H[i + 1][:, j0:je],
                            )
                # mixer contribution for this layer (interleaved accumulation)
                yt, n = ylast[i]
                nc.tensor.matmul(
                    pm[:, :],
                    WM[:, i * 8 : (i + 1) * 8],
                    yt[:, n - FRAME : n],
                    start=(i == 0),
                    stop=(i == N_LAYERS - 1),
                    skip_group_check=True,
                )

            out_sb = opool.tile([8, FRAME], _F32, tag="osb", name="osb")
            nc.scalar.activation(
                out_sb[:, :],
                pm[:, :],
                mybir.ActivationFunctionType.Identity,
                bias=MB[:, 0:1],
            )
            nc.sync.dma_start(d_out[:, :], out_sb[:, :])

    nc.compile()
    return nc


def _host_weights(c0_kernel, c_kernels, c_biases, io_kernels, io_biases,
                  mixer_kernel, mixer_bias):
    """Block-diagonal weight matrices + io-bias folding, shared by cores."""
    eye8 = np.eye(8, dtype=np.float32)
    # layer-0 stacked taps [24, 64]: rows 0:8 tap2, 8:16 tap1, 16:24 tap0
    w0 = np.concatenate(
        [np.kron(eye8, c0_kernel[k, 0, :][None, :]) for k in (2, 1, 0)], axis=0
    ).astype(np.float32)
    # layers 1..9: [128, 9*128]; per layer: cols 0:64 = [tap1; tap0]
    # stacked K=128, cols 64:128 = tap2 (rows 0:64)
    wc = np.zeros((128, 9 * 128), dtype=np.float32)
    for i in range(9):
        wc[0:64, i * 128 : i * 128 + 64] = np.kron(eye8, c_kernels[i, 1])
        wc[64:128, i * 128 : i * 128 + 64] = np.kron(eye8, c_kernels[i, 0])
        wc[0:64, i * 128 + 64 : i * 128 + 128] = np.kron(eye8, c_kernels[i, 2])
    # residual blocks: [64, 640]; cols 0:576 U_i blockdiag (rows 0:64),
    # cols 576:640 x-broadcast (rows 0:8)
    wr = np.zeros((64, 640), dtype=np.float32)
    for i in range(9):
        wr[:, i * 64 : (i + 1) * 64] = np.kron(eye8, io_kernels[i, 0])
    wr[0:8, 576:640] = np.kron(eye8, np.ones((1, 8), np.float32))
    # mixer: [64, 80]
    wm = np.concatenate(
        [
            np.kron(eye8, mixer_kernel[0, i * 8 : (i + 1) * 8, 0][:, None])
            for i in range(N_LAYERS)
        ],
        axis=1,
    ).astype(np.float32)
    # conv biases with io biases folded through the conv taps:
    # h'_i drops all accumulated io biases kappa_i; conv_i(h_i) =
    # conv_i(h'_i) + sum_k W_ik^T kappa_i  (constant per out-channel).
    cb = np.zeros((8, N_LAYERS), dtype=np.float64)
    kappa = np.zeros(8, dtype=np.float64)
    for i in range(N_LAYERS):
        if i == 0:
            adj = np.zeros(8)
        else:
            adj = np.einsum("kio,i->o", c_kernels[i - 1].astype(np.float64),
                            kappa)
        cb[:, i] = c_biases[i].astype(np.float64) + adj
        if i < N_LAYERS - 1:
            kappa = kappa + io_biases[i].astype(np.float64)
    cb = np.tile(cb.astype(np.float32), (8, 1))  # [64, 10]
    mb = np.full((8, 1), float(np.asarray(mixer_bias).reshape(-1)[0]), np.float32)
    return dict(w0=w0, wc=wc, wr=wr, wm=wm, cb=cb, mb=mb)


_NC_CACHE = None


def _get_nc():
    global _NC_CACHE
    if _NC_CACHE is None:
        _NC_CACHE = _build_program()
    return _NC_CACHE


def run(inputs, trace=False, **spmd_kwargs):
    """Run on 8 cores; returns (full_output [64,128], BassKernelResults)."""
    x = np.asarray(inputs["x"], dtype=np.float32)
    shared = _host_weights(
        np.asarray(inputs["c0_kernel"], np.float32),
        np.asarray(inputs["c_kernels"], np.float32),
        np.asarray(inputs["c_biases"], np.float32),
        np.asarray(inputs["io_kernels"], np.float32),
        np.asarray(inputs["io_biases"], np.float32),
        np.asarray(inputs["mixer_kernel"], np.float32),
        np.asarray(inputs["mixer_bias"], np.float32),
    )
    xw = np.ascontiguousarray(x[:, T - W_X :])  # [64, 2174]
    in_maps = []
    for c in range(N_CORES):
        m = dict(shared)
        m["xw"] = np.ascontiguousarray(xw[c * B_LOC : (c + 1) * B_LOC])
        in_maps.append(m)
    nc = _get_nc()
    res = run_bass_kernel_spmd(
        nc, in_maps, core_ids=list(range(N_CORES)), trace=trace, **spmd_kwargs
    )
    out = np.concatenate([res.results[c]["out"] for c in range(N_CORES)], axis=0)
    return out.astype(np.float32), res


def kernel(**inputs):
    out, _ = run(inputs, trace=False)
    return out


# revision 17
# speedup vs baseline: 1.1576x; 1.1576x over previous
"""Trainium2 Bass kernel for nn_AudioDeviceModel (dilated causal conv stack).

Strategy:
  - Data parallel: batch 64 sharded as 8 rows per core across 8 cores.
  - Only the last FRAME=128 timesteps are output; the receptive field of the
    10-layer dilated conv stack (dilations 1..512, K=3) is 2047, so only the
    last 2174 input samples matter.  Per-layer output windows shrink
    accordingly (W_Y below).
  - On-chip layout: partitions = (batch b in 0..8) x (channel c in 0..8) = 64
    partitions; time on the free axis.  Convs become PSUM-accumulated matmuls
    with host-built block-diagonal weights kron(eye(8), W).
  - All matmul operands use float32r (1 cycle/col on the PE at N>=256 vs 4+
    cycles for fp32; ~11-bit mantissa).  To stop rounding error compounding
    down the residual chain, h_i is carried in exact fp32 (H tensors, VectorE
    adds) and only rounded f32r copies feed the matmuls.  io biases are
    folded into later conv biases on the host so the residual add is a
    single 2-input DVE op.
  - Tap stacking: A_i [128, W_H[i]] f32r holds round(h_i) in rows 0:64 and
    the same values shifted right by dil in rows 64:128, so conv taps 0+1
    run as ONE K=128 matmul and tap 2 as a K=64 matmul (2N instead of 3N
    PE cycles per tile).  Layer 0 stacks all 3 taps of the 1-channel input
    into one K=24 matmul (x stored 3x with shifts in rows 0:24).
  - Residual: 1 matmul (U_i blockdiag, y -> psum) + DVE add with H_i ->
    H_{i+1}, then two DVE round-casts produce A_{i+1} rows 0:64 and the
    shifted dup rows 64:128.
  - Mixer: 10 accumulated [64,8]x[64,128] matmuls interleaved at each
    layer's end (skip_group_check) + bias.
"""

import sys

import numpy as np

try:
    import concourse.bass as bass
except ImportError:  # fresh environment without the site path
    sys.path.insert(0, "/opt/trn_rl_repo")
    import concourse.bass as bass

import concourse.tile as tile
from concourse import bacc, mybir
from concourse.bass_utils import run_bass_kernel_spmd

N_LAYERS = 10
FRAME = 128
B, T = 64, 4096
N_CORES = 8
B_LOC = B // N_CORES  # 8 batch rows per core
NT = 512  # time-tile (one PSUM bank of f32)

# per-layer dilations and windows
DIL = [2**i for i in range(N_LAYERS)]
W_Y = [0] * N_LAYERS  # output window of layer i (cols of y_i computed)
W_H = [0] * N_LAYERS  # input window of layer i (cols of h_i needed)
W_Y[N_LAYERS - 1] = FRAME
for _i in range(N_LAYERS - 1, -1, -1):
    W_H[_i] = W_Y[_i] + 2 * DIL[_i]
    if _i > 0:
        W_Y[_i - 1] = W_H[_i]
W_X = W_H[0]  # 2174

_F32 = mybir.dt.float32
_F32R = mybir.dt.float32r


def _tiles(wy):
    """End-aligned tiling: (start, size) pairs — ragged first tile, then
    512-wide tiles."""
    r = wy % NT
    starts = ([0] if r else []) + list(range(r, wy, NT))
    return [(s, (starts[k + 1] if k + 1 < len(starts) else wy) - s)
            for k, s in enumerate(starts)]


def _build_program():
    """Build the Bass program once; weights/x arrive as DRAM inputs."""
    nc = bacc.Bacc(
        "TRN2",
        target_bir_lowering=False,
        debug=False,
        enable_asserts=True,
        num_devices=N_CORES,
    )

    d_x = nc.dram_tensor("xw", [B_LOC, W_X], _F32, kind="ExternalInput").ap()
    d_w0 = nc.dram_tensor("w0", [24, 64], _F32, kind="ExternalInput").ap()
    d_wc = nc.dram_tensor("wc", [128, 9 * 128], _F32, kind="ExternalInput").ap()
    d_wr = nc.dram_tensor("wr", [64, 640], _F32, kind="ExternalInput").ap()
    d_wm = nc.dram_tensor("wm", [64, 80], _F32, kind="ExternalInput").ap()
    d_cb = nc.dram_tensor("cb", [64, N_LAYERS], _F32, kind="ExternalInput").ap()
    d_mb = nc.dram_tensor("mb", [8, 1], _F32, kind="ExternalInput").ap()
    d_out = nc.dram_tensor("out", [B_LOC, FRAME], _F32, kind="ExternalOutput").ap()

    with tile.TileContext(nc) as tc:
        with (
            tc.tile_pool(name="wpool", bufs=1) as wpool,
            tc.tile_pool(name="apool", bufs=2) as apool,
            tc.tile_pool(name="hpool", bufs=2) as hpool,
            tc.tile_pool(name="ypool", bufs=4) as ypool,
            tc.tile_pool(name="opool", bufs=1) as opool,
            tc.tile_pool(name="py", bufs=1, space="PSUM") as pyp,
            tc.tile_pool(name="ph", bufs=2, space="PSUM") as php,
            tc.tile_pool(name="pm", bufs=1, space="PSUM") as pmp,
        ):
            # --- inputs/weights: DMA to fp32 staging, round-copy to f32r.
            # Emission order matters: DVE runs casts in order, so cast the
            # things layer 0 needs FIRST (x, layer-0 taps, residual weights)
            # and the big conv-weight block last, overlapping compute.
            # x triplicated with shifts into rows 0:24 via 3 DMAs so conv0
            # is ONE K=24 matmul per tile (col j0+2+j: rows 0:8 tap2,
            # 8:16 tap1, 16:24 tap0)
            XS3 = opool.tile([24, W_X], _F32, tag="XS3", name="XS3")
            nc.sync.dma_start(XS3[0:8, :], d_x[:, :])
            nc.sync.dma_start(XS3[8:16, 1:W_X], d_x[:, 0 : W_X - 1])
            nc.sync.dma_start(XS3[16:24, 2:W_X], d_x[:, 0 : W_X - 2])
            WHs = wpool.tile([128, 192 + 9 * 128], _F32, tag="WHs", name="WHs")
            WH = wpool.tile([128, 192 + 9 * 128], _F32R, tag="WH", name="WH")
            nc.sync.dma_start(WHs[0:24, 0:64], d_w0[:, :])
            nc.vector.tensor_copy(WH[0:24, 0:64], WHs[0:24, 0:64])
            A0 = apool.tile([128, W_H[0]], _F32R, tag="A", name="A0")
            nc.vector.tensor_copy(A0[0:24, 2:W_X], XS3[:, 2:W_X])
            CB = wpool.tile([64, N_LAYERS], _F32, tag="CB", name="CB")
            nc.sync.dma_start(CB[:, :], d_cb[:, :])
            MB = wpool.tile([8, 1], _F32, tag="MB", name="MB")
            nc.sync.dma_start(MB[:, :], d_mb[:, :])
            # residual weights: cols 0:576 U_i blockdiag (rows 0:64);
            # cols 576:640 x-broadcast (rows 0:8)
            WRs = wpool.tile([64, 640], _F32, tag="WRs", name="WRs")
            WR = wpool.tile([64, 640], _F32R, tag="WR", name="WR")
            nc.sync.dma_start(WRs[:, :], d_wr[:, :])
            nc.vector.tensor_copy(WR[:, 0:576], WRs[:, 0:576])
            nc.vector.tensor_copy(WR[0:8, 576:640], WRs[0:8, 576:640])
            # big conv-weight block: split DMA across two queues
            half = 192 + 4 * 128
            nc.sync.dma_start(WHs[:, 192:half], d_wc[:, 0 : half - 192])
            nc.sync.dma_start(WHs[:, half:], d_wc[:, half - 192 :])
            nc.vector.tensor_copy(WH[:, 192:half], WHs[:, 192:half])
            nc.vector.tensor_copy(WH[:, half:], WHs[:, half:])
            # mixer weights rows 0:64
            WMs = wpool.tile([64, 80], _F32, tag="WMs", name="WMs")
            nc.sync.dma_start(WMs[:, :], d_wm[:, :])
            WM = wpool.tile([64, 80], _F32R, tag="WM", name="WM")
            nc.vector.tensor_copy(WM[:, :], WMs[:, :])

            # --- activation tensors ---
            # A_i [128, W_H[i]] f32r: rows 0:64 round(h_i); rows 64:128 the
            # same shifted right by DIL[i] (for the stacked taps-(1,0)
            # matmul).  A_0: x in rows 0:8, x>>1 rows 8:16, x>>2 rows 16:24.
            # H_i [64, W_H[i]] fp32 exact h chain.  A/H rotate (bufs=2).
            A = [None] * N_LAYERS
            H = [None] * N_LAYERS
            ylast = [None] * N_LAYERS
            A[0] = A0

            pm = pmp.tile([8, FRAME], _F32, tag="pm", name="pm")

            for i in range(N_LAYERS):
                d = DIL[i]
                wy = W_Y[i]
                off = 2 * d  # h_i col offset for time-aligned reads
                if i < N_LAYERS - 1:
                    A[i + 1] = apool.tile(
                        [128, W_H[i + 1]], _F32R, tag="A", name=f"A{i+1}"
                    )
                    H[i + 1] = hpool.tile(
                        [64, W_H[i + 1]], _F32, tag="H", name=f"H{i+1}"
                    )
                tl = _tiles(wy)
                # k-major matmul order: consecutive MMs share lhsT (the PE
                # pays ~150ns per weight swap), using len(tl)<=5 psum banks
                pys = [
                    pyp.tile([64, n], _F32, tag=f"py{ti}", name=f"py_{i}_{j0}")
                    for ti, (j0, n) in enumerate(tl)
                ]
                if i == 0:
                    for (j0, n), py in zip(tl, pys):
                        nc.tensor.matmul(
                            py[:, :],
                            WH[0:24, 0:64],
                            A[0][0:24, j0 + 2 : j0 + 2 + n],
                            start=True,
                            stop=True,
                        )
                else:
                    c0 = 192 + (i - 1) * 128
                    for (j0, n), py in zip(tl, pys):
                        # taps 1+0 stacked: rows 0:64 h at col+d (tap1),
                        # rows 64:128 dup(h>>d) at col+d = h at col (tap0)
                        nc.tensor.matmul(
                            py[:, :],
                            WH[0:128, c0 : c0 + 64],
                            A[i][0:128, j0 + d : j0 + d + n],
                            start=True,
                            stop=False,
                        )
                    for (j0, n), py in zip(tl, pys):
                        nc.tensor.matmul(
                            py[:, :],
                            WH[0:64, c0 + 64 : c0 + 128],
                            A[i][0:64, j0 + 2 * d : j0 + 2 * d + n],
                            start=False,
                            stop=True,
                        )
                # relu pass (ACT)
                yts = []
                for ti, ((j0, n), py) in enumerate(zip(tl, pys)):
                    ytag = f"YL{i}" if ti == len(tl) - 1 else "Y"
                    yt = ypool.tile([64, n], _F32R, tag=ytag, name=f"Y_{i}_{j0}")
                    nc.scalar.activation(
                        yt[:, :],
                        py[:, :],
                        mybir.ActivationFunctionType.Relu,
                        bias=CB[:, i : i + 1],
                    )
                    yts.append(yt)
                    ylast[i] = (yt, n)
                if i < N_LAYERS - 1:
                    # residual matmuls (same U_i weights back-to-back)
                    phs = []
                    for (j0, n), yt in zip(tl, yts):
                        ph = php.tile([64, n], _F32, tag="ph", name=f"ph_{i}_{j0}")
                        nc.tensor.matmul(
                            ph[:, :],
                            WR[0:64, i * 64 : i * 64 + 64],
                            yt[:, :],
                            start=True,
                            stop=(i != 0),
                            skip_group_check=True,
                        )
                        phs.append(ph)
                    if i == 0:
                        for (j0, n), ph in zip(tl, phs):
                            nc.tensor.matmul(
                                ph[:, :],
                                WR[0:8, 576:640],
                                A[0][0:8, off + j0 : off + j0 + n],
                                start=False,
                                stop=True,
                                skip_group_check=True,
                            )
                    # epilogue per tile: exact-fp32 h chain + f32r casts
                    d1 = DIL[i + 1]
                    for (j0, n), ph in zip(tl, phs):
                        if i == 0:
                            nc.vector.tensor_copy(H[1][:, j0 : j0 + n], ph[:, :])
                        else:
                            nc.vector.tensor_add(
                                H[i + 1][:, j0 : j0 + n],
                                ph[:, :],
                                H[i][:, off + j0 : off + j0 + n],
                            )
                        nc.vector.tensor_copy(
                            A[i + 1][0:64, j0 : j0 + n], H[i + 1][:, j0 : j0 + n]
                        )
                        je = min(j0 + n, W_Y[i + 1])
                        if j0 < je:
                            nc.vector.tensor_copy(
                                A[i + 1][64:128, j0 + d1 : je + d1],
                                H[i + 1][:, j0:je],
                            )
                # mixer contribution for this layer (interleaved accumulation)
                yt, n = ylast[i]
                nc.tensor.matmul(
                    pm[:, :],
                    WM[:, i * 8 : (i + 1) * 8],
                    yt[:, n - FRAME : n],
                    start=(i == 0),
                    stop=(i == N_LAYERS - 1),
                    skip_group_check=True,
                )

            out_sb = opool.tile([8, FRAME], _F32, tag="osb", name="osb")
            nc.scalar.activation(
                out_sb[:, :],
                pm[:, :],
                mybir.ActivationFunctionType.Identity,
                bias=MB[:, 0:1],
            )
            nc.sync.dma_start(d_out[:, :], out_sb[:, :])

    nc.compile()
    return nc


def _host_weights(c0_kernel, c_kernels, c_biases, io_kernels, io_biases,
                  mixer_kernel, mixer_bias):
    """Block-diagonal weight matrices + io-bias folding, shared by cores."""
    eye8 = np.eye(8, dtype=np.float32)
    # layer-0 stacked taps [24, 64]: rows 0:8 tap2, 8:16 tap1, 16:24 tap0
    w0 = np.concatenate(
        [np.kron(eye8, c0_kernel[k, 0, :][None, :]) for k in (2, 1, 0)], axis=0
    ).astype(np.float32)
    # layers 1..9: [128, 9*128]; per layer: cols 0:64 = [tap1; tap0]
    # stacked K=128, cols 64:128 = tap2 (rows 0:64)
    wc = np.zeros((128, 9 * 128), dtype=np.float32)
    for i in range(9):
        wc[0:64, i * 128 : i * 128 + 64] = np.kron(eye8, c_kernels[i, 1])
        wc[64:128, i * 128 : i * 128 + 64] = np.kron(eye8, c_kernels[i, 0])
        wc[0:64, i * 128 + 64 : i * 128 + 128] = np.kron(eye8, c_kernels[i, 2])
    # residual blocks: [64, 640]; cols 0:576 U_i blockdiag (rows 0:64),
    # cols 576:640 x-broadcast (rows 0:8)
    wr = np.zeros((64, 640), dtype=np.float32)
    for i in range(9):
        wr[:, i * 64 : (i + 1) * 64] = np.kron(eye8, io_kernels[i, 0])
    wr[0:8, 576:640] = np.kron(eye8, np.ones((1, 8), np.float32))
    # mixer: [64, 80]
    wm = np.concatenate(
        [
            np.kron(eye8, mixer_kernel[0, i * 8 : (i + 1) * 8, 0][:, None])
            for i in range(N_LAYERS)
        ],
        axis=1,
    ).astype(np.float32)
    # conv biases with io biases folded through the conv taps:
    # h'_i drops all accumulated io biases kappa_i; conv_i(h_i) =
    # conv_i(h'_i) + sum_k W_ik^T kappa_i  (constant per out-channel).
    cb = np.zeros((8, N_LAYERS), dtype=np.float64)
    kappa = np.zeros(8, dtype=np.float64)
    for i in range(N_LAYERS):
        if i == 0:
            adj = np.zeros(8)
        else:
            adj = np.einsum("kio,i->o", c_kernels[i - 1].astype(np.float64),
                            kappa)
        cb[:, i] = c_biases[i].astype(np.float64) + adj
        if i < N_LAYERS - 1:
            kappa = kappa + io_biases[i].astype(np.float64)
    cb = np.tile(cb.astype(np.float32), (8, 1))  # [64, 10]
    mb = np.full((8, 1), float(np.asarray(mixer_bias).reshape(-1)[0]), np.float32)
    return dict(w0=w0, wc=wc, wr=wr, wm=wm, cb=cb, mb=mb)


_NC_CACHE = None


def _get_nc():
    global _NC_CACHE
    if _NC_CACHE is None:
        _NC_CACHE = _build_program()
    return _NC_CACHE


def run(inputs, trace=False, **spmd_kwargs):
    """Run on 8 cores; returns (full_output [64,128], BassKernelResults)."""
    x = np.asarray(inputs["x"], dtype=np.float32)
    shared = _host_weights(
        np.asarray(inputs["c0_kernel"], np.float32),
        np.asarray(inputs["c_kernels"], np.float32),
        np.asarray(inputs["c_biases"], np.float32),
        np.asarray(inputs["io_kernels"], np.float32),
        np.asarray(inputs["io_biases"], np.float32),
        np.asarray(inputs["mixer_kernel"], np.float32),
        np.asarray(inputs["mixer_bias"], np.float32),
    )
    xw = np.ascontiguousarray(x[:, T - W_X :])  # [64, 2174]
    in_maps = []
    for c in range(N_CORES):
        m = dict(shared)
        m["xw"] = np.ascontiguousarray(xw[c * B_LOC : (c + 1) * B_LOC])
        in_maps.append(m)
    nc = _get_nc()
    res = run_bass_kernel_spmd(
        nc, in_maps, core_ids=list(range(N_CORES)), trace=trace, **spmd_kwargs
    )
    out = np.concatenate([res.results[c]["out"] for c in range(N_CORES)], axis=0)
    return out.astype(np.float32), res


def kernel(**inputs):
    out, _ = run(inputs, trace=False)
    return out
